# revision 30
# baseline (speedup 1.0000x reference)
"""Trainium2 Bass kernel for nn_AlignmentMatrix.

score[b,i,j] = [ctx_i ; asp_j ; ctx_i*asp_j] @ w_u
            = sum_d ctx[b,i,d]*w3[d]*asp[b,j,d] + ctx[b]@w1 + asp[b]@w2

Reformulated per batch as a single matmul over host-marshalled operands:
    out[b] = ctxp[b] @ R[b]
with (D=400)
    ctxp[b][i, 0:400] = ctx[b][i, :]           (fp16)
    ctxp[b][i, 400]   = 1.0                     (bias lane)
    R[b][d, j] = w3[d]*asp[b,j,d] + w1[d]       (folds ctx@w1)
    R[b][400, j] = asp[b,j,:] @ w2              (folds asp@w2)
The 54.9 GFLOP contraction runs on-device with fp32 PSUM accumulation;
host prep is O(B*L*D) elementwise marshalling + layout.

Marshalling / kernel-structure choices (trace-driven):
  - ctx ships ALREADY TRANSPOSED (d-major): 3 full K=128 blocks
    [pb, 384, 2048]; the contraction tail (16 dims + bias lane, K=17)
    ships separately, replicated at partition offsets {0,32,64,96}:
    the four row-slots of an output group run their tail matmuls
    CONCURRENTLY in one array pass via tile_position row tiling.
  - the i-axis is permuted host-side as i' = g*512 + r*128 + p
    (i = g*512 + 4p + r) so output partition p holds 4 CONSECUTIVE
    output rows -> every store descriptor is one 4KB contiguous line.
  - PAIRED TAIL BUNDLES: groups run in pairs as
    [12 fulls g0][bundle g0][bundle g1][12 fulls g1].  A quadrant
    LDWEIGHTS cannot be pulled ahead of an in-flight full-array matmul
    (and vice versa), so each fulls<->bundle adjacency exposes ~100ns
    of weight-load; pairing gives 3 boundaries per 2 groups instead
    of 4 (measured -110ns/group).
  - per-batch pool tiles (bufs=2-3) keep each DMA's consumer set to
    one batch: the Tile framework recycles its 8 DMA semaphore lanes
    with waits on the previous lane user's consumers, so big
    long-lived tiles serialize loads behind whole batches (v2 lesson).
  - pools are seal()ed, not release()d: release emits sync deps on
    every pool user which all materialize as an end-of-kernel
    semaphore drain.
  - batch-0's critical operands ride THREE parallel issue queues
    (sync/HWDGE: dup'd group-0 ctx cols + tail R; scalar/HWDGE: R
    blocks + tails; gpsimd/SWDGE: ctx remainders) because each
    dma_start costs its engine ~650ns of descriptor time; a warmup
    burst sized to ABUT the first data-ready matmul covers the fixed
    ~7us engine preamble + load head with zero PE gap -- any
    warmup->stream gap resets the HAM activity window and costs
    ~2-3us of half-clock matmuls.
  - ot rides a 4-deep ring so the store-completion WAR never parks at
    the copy engines' strict-FIFO heads (3-deep measurably stalled
    the PE at pair boundaries via psum-bank reuse).
  - the last pair drains with per-slot copies and per-slot stores.

Device pipeline per batch: bulk DMA loads (ctx as one 3D transfer),
2 group-pairs of matmuls -> PSUM, PSUM->SBUF fp16 copies alternating
scalar/vector engines, 4KB-per-partition stores.  The PE runs nothing
but matmuls, back-to-back.  Host upcasts fp16 -> f32.
"""

import numpy as np

import concourse.bass as bass
from concourse import bacc
import concourse.mybir as mybir
import concourse.tile as tile
from concourse.bass_utils import run_bass_kernel_spmd

F32 = mybir.dt.float32
F16 = mybir.dt.float16

B, LC, LA, D = 64, 2048, 512, 400
NCH = 3           # full K=128 blocks; tail handled by the bundle
KT = 17           # tail rows: 16 data dims + bias lane
N_CORES = 8
PB = B // N_CORES  # batches per core
P = 128
RSLOT = 4          # consecutive out rows per partition
GROUP = P * RSLOT  # out rows per group (512)
NG = LC // GROUP   # groups per batch
N_WARM = 44


def build_kernel(pb: int = PB, lc: int = LC) -> bass.Bass:
    nc = bacc.Bacc(
        "TRN2",
        target_bir_lowering=False,
        debug=False,
        num_devices=N_CORES,
    )
    ctx_d = nc.dram_tensor("ctx", [pb, NCH * P, lc], F16, kind="ExternalInput").ap()
    tl_d = nc.dram_tensor("tl", [pb, P, NG, P], F16, kind="ExternalInput").ap()
    rr_d = nc.dram_tensor("rr", [pb, P, NCH + 1, LA], F16, kind="ExternalInput").ap()
    # batch-0 group-0 ctx columns duplicated partition-major: one DMA's
    # worth of critical head data (the host ships 0.4MB twice for b0),
    # flat so the transfer runs as 3KB-contiguous-per-partition
    # descriptors instead of 3x1KB
    c0_d = nc.dram_tensor("c0", [P, NCH * GROUP], F16, kind="ExternalInput").ap()
    out_d = nc.dram_tensor("out", [pb, lc, LA], F16, kind="ExternalOutput").ap()

    with tile.TileContext(nc) as tc:
        _kernel_body(tc, out_d, ctx_d, tl_d, rr_d, c0_d, pb, lc)
    nc.compile()
    return nc


def _kernel_body(tc, out_d, ctx_d, tl_d, rr_d, c0_d, pb, lc):
    nc = tc.nc

    ctx_pool = tc.alloc_tile_pool(name="ctxT", bufs=3)
    tl_pool = tc.alloc_tile_pool(name="tl", bufs=2)
    rr_pool = tc.alloc_tile_pool(name="rrt", bufs=3)
    out_pool = tc.alloc_tile_pool(name="outT", bufs=4)
    warm_pool = tc.alloc_tile_pool(name="warm", bufs=1)
    psum_o = tc.alloc_tile_pool(name="psumO", bufs=8, space="PSUM")

    # HAM warm-up: dummy matmuls with no input dependencies run during
    # the engine preamble + input-DMA head, so the PE is near K=8/8
    # when the real stream starts.  Output is a scratch PSUM bank that
    # the ring recycles with an ordinary WAW dependency.
    # K=128 operands: a K=1 warmup matmul lights up only one array row
    # and the HAM activity monitor never trips (every trace showed K=8/8
    # firing ~3.4us after the REAL stream, not after the warmup) -- full
    # contraction depth at the same N=128 pacing warms the clock gate
    # during the DMA head instead
    wl = warm_pool.tile([P, P], F16, tag="wl", name="wl")
    wr = warm_pool.tile([P, P], F16, tag="wr", name="wr")
    # dedicated landing tile for batch-0 group-0 ctx columns: contiguous
    # per-partition destination -> big descriptors -> earliest possible
    # completion of the stream-gating DMA
    c0t = warm_pool.tile([P, NCH * GROUP], F16, tag="c0t", name="c0t")
    # gpsimd: its engine preamble retires ~1.5us before vector's, so the
    # warmup matmuls (which wait on these) start that much earlier
    nc.gpsimd.memset(wl, 0.0)
    nc.gpsimd.memset(wr, 0.0)
    pw = psum_o.tile([P, LA], F32, tag="pO", name="pw")
    for i in range(N_WARM):
        nc.tensor.matmul(pw[:, 0:P], wl, wr, start=True, stop=True)

    copy_parity = 0

    def copy(dst, src):
        # alternating engines: cheapest in total engine time (one
        # instruction per bank); the 4-deep ot ring keeps the store
        # WAR off the copy critical path
        nonlocal copy_parity
        if copy_parity & 1:
            nc.vector.tensor_copy(dst, src)
        else:
            nc.scalar.copy(dst, src)
        copy_parity += 1

    def store(b, g, ott):
        dst = out_d[b, g * GROUP : (g + 1) * GROUP, :].rearrange(
            "(p r) j -> p r j", p=P
        )
        src = ott.rearrange("p (r j) -> p r j", r=RSLOT)
        nc.sync.dma_start(out=dst, in_=src)

    for b in range(pb):
        rrt = rr_pool.tile([P, (NCH + 1) * LA], F16, tag="rrt", name=f"rrt_{b}")
        tlt = tl_pool.tile([P, NG * P], F16, tag="tl", name=f"tl_{b}")
        ctxT = ctx_pool.tile([P, NCH * lc], F16, tag="ctxT", name=f"ctxT_{b}")

        if b == 0:
            # first batch rides THREE parallel issue queues (each
            # dma_start costs its engine ~650ns of descriptor time):
            # sync gets R, scalar gets the duplicated group-0 ctx
            # columns + tails, gpsimd streams the ctx remainders.
            nc.sync.dma_start(out=c0t, in_=c0_d)
            nc.sync.dma_start(
                out=rrt[:, NCH * LA : (NCH + 1) * LA],
                in_=rr_d[b, :, NCH, :],
            )
            nc.scalar.dma_start(
                out=rrt.rearrange("p (c j) -> p c j", c=NCH + 1)[:, 0:NCH],
                in_=rr_d[b, :, 0:NCH, :],
            )
            nc.scalar.dma_start(
                out=tlt.rearrange("p (g q) -> p g q", g=NG),
                in_=tl_d[b],
            )
            for c in range(NCH):
                nc.gpsimd.dma_start(
                    out=ctxT[:, c * lc + GROUP : (c + 1) * lc],
                    in_=ctx_d[b, c * P : (c + 1) * P, GROUP:],
                )
        else:
            nc.gpsimd.dma_start(
                out=rrt.rearrange("p (c j) -> p c j", c=NCH + 1),
                in_=rr_d[b],
            )
            nc.gpsimd.dma_start(
                out=tlt.rearrange("p (g q) -> p g q", g=NG),
                in_=tl_d[b],
            )
            nc.gpsimd.dma_start(
                out=ctxT.rearrange("p (c x) -> p c x", c=NCH),
                in_=ctx_d[b].rearrange("(c p) x -> p c x", c=NCH),
            )

        last_b = b == pb - 1
        for pair in range(NG // 2):
            g0, g1 = 2 * pair, 2 * pair + 1
            o0 = out_pool.tile([P, RSLOT * LA], F16, tag="ot",
                               name=f"ot_{b}_{g0}")
            o1 = out_pool.tile([P, RSLOT * LA], F16, tag="ot",
                               name=f"ot_{b}_{g1}")
            ps0 = [psum_o.tile([P, LA], F32, tag="pO", name=f"pO_{b}_{g0}_{r}")
                   for r in range(RSLOT)]
            ps1 = [psum_o.tile([P, LA], F32, tag="pO", name=f"pO_{b}_{g1}_{r}")
                   for r in range(RSLOT)]
            last_pair = last_b and pair == NG // 2 - 1

            # 12 fulls of g0 (c==0 clears the banks); the very first
            # group reads from the dedicated head-landing tile
            head = b == 0 and pair == 0
            for r in range(RSLOT):
                for c in range(NCH):
                    if head:
                        lhs = c0t[:, c * GROUP + r * P : c * GROUP + (r + 1) * P]
                    else:
                        col = c * lc + g0 * GROUP + r * P
                        lhs = ctxT[:, col : col + P]
                    nc.tensor.matmul(
                        ps0[r],
                        lhs,
                        rrt[:, c * LA : (c + 1) * LA],
                        start=(c == 0),
                        stop=False,
                    )
            # bundle g0 closes its banks; g0 drains while g1 computes
            for r in range(RSLOT):
                nc.tensor.matmul(
                    ps0[r],
                    tlt[32 * r : 32 * r + KT, g0 * P : (g0 + 1) * P],
                    rrt[32 * r : 32 * r + KT, NCH * LA : (NCH + 1) * LA],
                    start=False,
                    stop=True,
                    tile_position=(32 * r, 0),
                )
            for r in range(RSLOT):
                copy(o0[:, r * LA : (r + 1) * LA], ps0[r])
            store(b, g0, o0)

            # bundle g1 opens its banks
            for r in range(RSLOT):
                nc.tensor.matmul(
                    ps1[r],
                    tlt[32 * r : 32 * r + KT, g1 * P : (g1 + 1) * P],
                    rrt[32 * r : 32 * r + KT, NCH * LA : (NCH + 1) * LA],
                    start=True,
                    stop=False,
                    tile_position=(32 * r, 0),
                )
            # 12 fulls of g1, per-slot close + copy for fast drain
            dst1 = out_d[b, g1 * GROUP : (g1 + 1) * GROUP, :].rearrange(
                "(p r) j -> p r j", p=P
            )
            src1 = o1.rearrange("p (r j) -> p r j", r=RSLOT)
            for r in range(RSLOT):
                for c in range(NCH):
                    col = c * lc + g1 * GROUP + r * P
                    nc.tensor.matmul(
                        ps1[r],
                        ctxT[:, col : col + P],
                        rrt[:, c * LA : (c + 1) * LA],
                        start=False,
                        stop=(c == NCH - 1),
                    )
                copy(o1[:, r * LA : (r + 1) * LA], ps1[r])
                if last_pair:
                    # per-slot final stores: each issues as its slot's
                    # copy completes, so the drain overlaps the stream
                    nc.sync.dma_start(out=dst1[:, r : r + 1, :],
                                      in_=src1[:, r : r + 1, :])
            if not last_pair:
                store(b, g1, o1)

    # seal (NOT release): release emits sync deps on every pool user,
    # which all land as an end-of-kernel semaphore drain.
    for p in (ctx_pool, tl_pool, rr_pool, out_pool, warm_pool, psum_o):
        p.seal()


def _prep_inputs(ctx, asp, w_u):
    """Host-side marshalling: fp16 cast, transpose/permute, R formation."""
    ctx = np.asarray(ctx, dtype=np.float32)
    asp = np.asarray(asp, dtype=np.float32)
    w = np.asarray(w_u, dtype=np.float32).reshape(-1)
    w1, w2, w3 = w[:D], w[D : 2 * D], w[2 * D :]

    # ctxT (first 384 dims) with i' = g*512 + r*128 + p <-> i = g*512+4p+r
    # [B, i, d] -> [B, d, g, p, r] -> [B, d, g, r, p]
    cr = ctx.reshape(B, NG, P, RSLOT, D)
    ctxp = (
        np.transpose(cr[..., : NCH * P], (0, 4, 1, 3, 2))
        .reshape(B, NCH * P, LC)
        .astype(np.float16)
    )

    # tails: [B, 128, g, p]; partition 32r+t holds tail dim t (t<16) or
    # the bias lane (t=16) for slot r; column (g, p) is out row g*512+4p+r
    tails = np.zeros((B, P, NG, P), dtype=np.float16)
    tail_d = np.transpose(cr[..., NCH * P :], (0, 4, 1, 2, 3))  # [B,16,g,p,r]
    for r in range(RSLOT):
        tails[:, 32 * r : 32 * r + 16, :, :] = tail_d[..., r]
        tails[:, 32 * r + 16, :, :] = 1.0

    # R[b, dd, c, j]: blocks 0..2 rows dd -> d = 128c + dd; block 3 rows
    # 32r+t -> tail slice + asp_term row (replicated per slot offset)
    scaled = (asp * w3[None, None, :] + w1[None, None, :]).transpose(0, 2, 1)
    at = asp @ w2
    rr = np.zeros((B, P, NCH + 1, LA), dtype=np.float16)
    for c in range(NCH):
        rr[:, :, c, :] = scaled[:, P * c : P * (c + 1), :]
    for r in range(RSLOT):
        rr[:, 32 * r : 32 * r + 16, NCH, :] = scaled[:, NCH * P :, :]
        rr[:, 32 * r + 16, NCH, :] = at
    return ctxp, tails, rr


def kernel(batch_size=None, ctx=None, asp=None, w_u=None, **_unused):
    ctxp, tails, rr = _prep_inputs(ctx, asp, w_u)

    nc = build_kernel()
    in_maps = [
        {
            "ctx": ctxp[i * PB : (i + 1) * PB],
            "tl": tails[i * PB : (i + 1) * PB],
            "rr": rr[i * PB : (i + 1) * PB],
            # batch-0 group-0 ctx columns, partition-major dup for the head
            "c0": np.ascontiguousarray(
                ctxp[i * PB, :, 0:GROUP]
                .reshape(NCH, P, GROUP)
                .transpose(1, 0, 2)
            ).reshape(P, NCH * GROUP),
        }
        for i in range(N_CORES)
    ]
    res = run_bass_kernel_spmd(
        nc, in_maps, core_ids=list(range(N_CORES)), **_RUN_KWARGS
    )
    _LAST_RESULTS.clear()
    _LAST_RESULTS.append(res)
    # stores write natural row order (partition p, slot r -> row 4p+r)
    out = np.concatenate(
        [np.asarray(res.results[i]["out"]) for i in range(N_CORES)], axis=0
    )
    return out.astype(np.float32)


# test-harness hooks: extra kwargs for run_bass_kernel_spmd (e.g. trace=True)
# and the last BassKernelResults for profiling. Unused in grading.
_RUN_KWARGS: dict = {}
_LAST_RESULTS: list = []


# revision 31
# speedup vs baseline: 1.0008x; 1.0008x over previous
"""Trainium2 Bass kernel for nn_AlignmentMatrix.

score[b,i,j] = [ctx_i ; asp_j ; ctx_i*asp_j] @ w_u
            = sum_d ctx[b,i,d]*w3[d]*asp[b,j,d] + ctx[b]@w1 + asp[b]@w2

Reformulated per batch as a single matmul over host-marshalled operands:
    out[b] = ctxp[b] @ R[b]
with (D=400)
    ctxp[b][i, 0:400] = ctx[b][i, :]           (fp16)
    ctxp[b][i, 400]   = 1.0                     (bias lane)
    R[b][d, j] = w3[d]*asp[b,j,d] + w1[d]       (folds ctx@w1)
    R[b][400, j] = asp[b,j,:] @ w2              (folds asp@w2)
The 54.9 GFLOP contraction runs on-device with fp32 PSUM accumulation;
host prep is O(B*L*D) elementwise marshalling + layout.

Marshalling / kernel-structure choices (trace-driven):
  - ctx ships ALREADY TRANSPOSED (d-major): 3 full K=128 blocks
    [pb, 384, 2048]; the contraction tail (16 dims + bias lane, K=17)
    ships separately, replicated at partition offsets {0,32,64,96}:
    the four row-slots of an output group run their tail matmuls
    CONCURRENTLY in one array pass via tile_position row tiling.
  - the i-axis is permuted host-side as i' = g*512 + r*128 + p
    (i = g*512 + 4p + r) so output partition p holds 4 CONSECUTIVE
    output rows -> every store descriptor is one 4KB contiguous line.
  - PAIRED TAIL BUNDLES: groups run in pairs as
    [12 fulls g0][bundle g0][bundle g1][12 fulls g1].  A quadrant
    LDWEIGHTS cannot be pulled ahead of an in-flight full-array matmul
    (and vice versa), so each fulls<->bundle adjacency exposes ~100ns
    of weight-load; pairing gives 3 boundaries per 2 groups instead
    of 4 (measured -110ns/group).
  - per-batch pool tiles (bufs=2-3) keep each DMA's consumer set to
    one batch: the Tile framework recycles its 8 DMA semaphore lanes
    with waits on the previous lane user's consumers, so big
    long-lived tiles serialize loads behind whole batches (v2 lesson).
  - pools are seal()ed, not release()d: release emits sync deps on
    every pool user which all materialize as an end-of-kernel
    semaphore drain.
  - batch-0's critical operands ride THREE parallel issue queues
    (sync/HWDGE: dup'd group-0 ctx cols + tail R; scalar/HWDGE: R
    blocks + tails; gpsimd/SWDGE: ctx remainders) because each
    dma_start costs its engine ~650ns of descriptor time; a warmup
    burst sized to ABUT the first data-ready matmul covers the fixed
    ~7us engine preamble + load head with zero PE gap -- any
    warmup->stream gap resets the HAM activity window and costs
    ~2-3us of half-clock matmuls.
  - ot rides a 4-deep ring so the store-completion WAR never parks at
    the copy engines' strict-FIFO heads (3-deep measurably stalled
    the PE at pair boundaries via psum-bank reuse).
  - the last pair drains with per-slot copies and per-slot stores.

Device pipeline per batch: bulk DMA loads (ctx as one 3D transfer),
2 group-pairs of matmuls -> PSUM, PSUM->SBUF fp16 copies alternating
scalar/vector engines, 4KB-per-partition stores.  The PE runs nothing
but matmuls, back-to-back.  Host upcasts fp16 -> f32.
"""

import numpy as np

import concourse.bass as bass
from concourse import bacc
import concourse.mybir as mybir
import concourse.tile as tile
from concourse.bass_utils import run_bass_kernel_spmd

F32 = mybir.dt.float32
F16 = mybir.dt.float16

B, LC, LA, D = 64, 2048, 512, 400
NCH = 3           # full K=128 blocks; tail handled by the bundle
KT = 17           # tail rows: 16 data dims + bias lane
N_CORES = 8
PB = B // N_CORES  # batches per core
P = 128
RSLOT = 4          # consecutive out rows per partition
GROUP = P * RSLOT  # out rows per group (512)
NG = LC // GROUP   # groups per batch
N_WARM = 42


def build_kernel(pb: int = PB, lc: int = LC) -> bass.Bass:
    nc = bacc.Bacc(
        "TRN2",
        target_bir_lowering=False,
        debug=False,
        num_devices=N_CORES,
    )
    ctx_d = nc.dram_tensor("ctx", [pb, NCH * P, lc], F16, kind="ExternalInput").ap()
    tl_d = nc.dram_tensor("tl", [pb, P, NG, P], F16, kind="ExternalInput").ap()
    rr_d = nc.dram_tensor("rr", [pb, P, NCH + 1, LA], F16, kind="ExternalInput").ap()
    # batch-0 group-0 ctx columns duplicated partition-major: one DMA's
    # worth of critical head data (the host ships 0.4MB twice for b0),
    # flat so the transfer runs as 3KB-contiguous-per-partition
    # descriptors instead of 3x1KB
    c0_d = nc.dram_tensor("c0", [P, NCH * GROUP], F16, kind="ExternalInput").ap()
    out_d = nc.dram_tensor("out", [pb, lc, LA], F16, kind="ExternalOutput").ap()

    with tile.TileContext(nc) as tc:
        _kernel_body(tc, out_d, ctx_d, tl_d, rr_d, c0_d, pb, lc)
    nc.compile()
    return nc


def _kernel_body(tc, out_d, ctx_d, tl_d, rr_d, c0_d, pb, lc):
    nc = tc.nc

    ctx_pool = tc.alloc_tile_pool(name="ctxT", bufs=3)
    tl_pool = tc.alloc_tile_pool(name="tl", bufs=2)
    rr_pool = tc.alloc_tile_pool(name="rrt", bufs=3)
    out_pool = tc.alloc_tile_pool(name="outT", bufs=4)
    warm_pool = tc.alloc_tile_pool(name="warm", bufs=1)
    psum_o = tc.alloc_tile_pool(name="psumO", bufs=8, space="PSUM")

    # HAM warm-up: dummy matmuls with no input dependencies run during
    # the engine preamble + input-DMA head, so the PE is near K=8/8
    # when the real stream starts.  Output is a scratch PSUM bank that
    # the ring recycles with an ordinary WAW dependency.
    # K=128 operands: a K=1 warmup matmul lights up only one array row
    # and the HAM activity monitor never trips (every trace showed K=8/8
    # firing ~3.4us after the REAL stream, not after the warmup) -- full
    # contraction depth at the same N=128 pacing warms the clock gate
    # during the DMA head instead
    wl = warm_pool.tile([P, P], F16, tag="wl", name="wl")
    wr = warm_pool.tile([P, P], F16, tag="wr", name="wr")
    # dedicated landing tile for batch-0 group-0 ctx columns: contiguous
    # per-partition destination -> big descriptors -> earliest possible
    # completion of the stream-gating DMA
    c0t = warm_pool.tile([P, NCH * GROUP], F16, tag="c0t", name="c0t")
    # gpsimd: its engine preamble retires ~1.5us before vector's, so the
    # warmup matmuls (which wait on these) start that much earlier
    nc.gpsimd.memset(wl, 0.0)
    nc.gpsimd.memset(wr, 0.0)
    pw = psum_o.tile([P, LA], F32, tag="pO", name="pw")
    for i in range(N_WARM):
        nc.tensor.matmul(pw[:, 0:P], wl, wr, start=True, stop=True)

    copy_parity = 0

    def copy(dst, src):
        # alternating engines: cheapest in total engine time (one
        # instruction per bank); the 4-deep ot ring keeps the store
        # WAR off the copy critical path
        nonlocal copy_parity
        if copy_parity & 1:
            nc.vector.tensor_copy(dst, src)
        else:
            nc.scalar.copy(dst, src)
        copy_parity += 1

    def store(b, g, ott):
        dst = out_d[b, g * GROUP : (g + 1) * GROUP, :].rearrange(
            "(p r) j -> p r j", p=P
        )
        src = ott.rearrange("p (r j) -> p r j", r=RSLOT)
        nc.sync.dma_start(out=dst, in_=src)

    for b in range(pb):
        rrt = rr_pool.tile([P, (NCH + 1) * LA], F16, tag="rrt", name=f"rrt_{b}")
        tlt = tl_pool.tile([P, NG * P], F16, tag="tl", name=f"tl_{b}")
        ctxT = ctx_pool.tile([P, NCH * lc], F16, tag="ctxT", name=f"ctxT_{b}")

        if b == 0:
            # first batch rides THREE parallel issue queues (each
            # dma_start costs its engine ~650ns of descriptor time):
            # sync gets R, scalar gets the duplicated group-0 ctx
            # columns + tails, gpsimd streams the ctx remainders.
            nc.sync.dma_start(out=c0t, in_=c0_d)
            nc.sync.dma_start(
                out=rrt[:, NCH * LA : (NCH + 1) * LA],
                in_=rr_d[b, :, NCH, :],
            )
            nc.scalar.dma_start(
                out=rrt.rearrange("p (c j) -> p c j", c=NCH + 1)[:, 0:NCH],
                in_=rr_d[b, :, 0:NCH, :],
            )
            nc.scalar.dma_start(
                out=tlt.rearrange("p (g q) -> p g q", g=NG),
                in_=tl_d[b],
            )
            for c in range(NCH):
                nc.gpsimd.dma_start(
                    out=ctxT[:, c * lc + GROUP : (c + 1) * lc],
                    in_=ctx_d[b, c * P : (c + 1) * P, GROUP:],
                )
        else:
            nc.gpsimd.dma_start(
                out=rrt.rearrange("p (c j) -> p c j", c=NCH + 1),
                in_=rr_d[b],
            )
            nc.gpsimd.dma_start(
                out=tlt.rearrange("p (g q) -> p g q", g=NG),
                in_=tl_d[b],
            )
            nc.gpsimd.dma_start(
                out=ctxT.rearrange("p (c x) -> p c x", c=NCH),
                in_=ctx_d[b].rearrange("(c p) x -> p c x", c=NCH),
            )

        last_b = b == pb - 1
        for pair in range(NG // 2):
            g0, g1 = 2 * pair, 2 * pair + 1
            o0 = out_pool.tile([P, RSLOT * LA], F16, tag="ot",
                               name=f"ot_{b}_{g0}")
            o1 = out_pool.tile([P, RSLOT * LA], F16, tag="ot",
                               name=f"ot_{b}_{g1}")
            ps0 = [psum_o.tile([P, LA], F32, tag="pO", name=f"pO_{b}_{g0}_{r}")
                   for r in range(RSLOT)]
            ps1 = [psum_o.tile([P, LA], F32, tag="pO", name=f"pO_{b}_{g1}_{r}")
                   for r in range(RSLOT)]
            last_pair = last_b and pair == NG // 2 - 1

            # 12 fulls of g0 (c==0 clears the banks); the very first
            # group reads from the dedicated head-landing tile
            head = b == 0 and pair == 0
            for r in range(RSLOT):
                for c in range(NCH):
                    if head:
                        lhs = c0t[:, c * GROUP + r * P : c * GROUP + (r + 1) * P]
                    else:
                        col = c * lc + g0 * GROUP + r * P
                        lhs = ctxT[:, col : col + P]
                    nc.tensor.matmul(
                        ps0[r],
                        lhs,
                        rrt[:, c * LA : (c + 1) * LA],
                        start=(c == 0),
                        stop=False,
                    )
            # bundle g0 closes its banks; g0 drains while g1 computes
            for r in range(RSLOT):
                nc.tensor.matmul(
                    ps0[r],
                    tlt[32 * r : 32 * r + KT, g0 * P : (g0 + 1) * P],
                    rrt[32 * r : 32 * r + KT, NCH * LA : (NCH + 1) * LA],
                    start=False,
                    stop=True,
                    tile_position=(32 * r, 0),
                )
            for r in range(RSLOT):
                copy(o0[:, r * LA : (r + 1) * LA], ps0[r])
            store(b, g0, o0)

            # bundle g1 opens its banks
            for r in range(RSLOT):
                nc.tensor.matmul(
                    ps1[r],
                    tlt[32 * r : 32 * r + KT, g1 * P : (g1 + 1) * P],
                    rrt[32 * r : 32 * r + KT, NCH * LA : (NCH + 1) * LA],
                    start=True,
                    stop=False,
                    tile_position=(32 * r, 0),
                )
            # 12 fulls of g1, per-slot close + copy for fast drain
            dst1 = out_d[b, g1 * GROUP : (g1 + 1) * GROUP, :].rearrange(
                "(p r) j -> p r j", p=P
            )
            src1 = o1.rearrange("p (r j) -> p r j", r=RSLOT)
            for r in range(RSLOT):
                for c in range(NCH):
                    col = c * lc + g1 * GROUP + r * P
                    nc.tensor.matmul(
                        ps1[r],
                        ctxT[:, col : col + P],
                        rrt[:, c * LA : (c + 1) * LA],
                        start=False,
                        stop=(c == NCH - 1),
                    )
                copy(o1[:, r * LA : (r + 1) * LA], ps1[r])
                if last_pair:
                    # per-slot final stores: each issues as its slot's
                    # copy completes, so the drain overlaps the stream
                    nc.sync.dma_start(out=dst1[:, r : r + 1, :],
                                      in_=src1[:, r : r + 1, :])
            if not last_pair:
                store(b, g1, o1)

    # seal (NOT release): release emits sync deps on every pool user,
    # which all land as an end-of-kernel semaphore drain.
    for p in (ctx_pool, tl_pool, rr_pool, out_pool, warm_pool, psum_o):
        p.seal()


def _prep_inputs(ctx, asp, w_u):
    """Host-side marshalling: fp16 cast, transpose/permute, R formation."""
    ctx = np.asarray(ctx, dtype=np.float32)
    asp = np.asarray(asp, dtype=np.float32)
    w = np.asarray(w_u, dtype=np.float32).reshape(-1)
    w1, w2, w3 = w[:D], w[D : 2 * D], w[2 * D :]

    # ctxT (first 384 dims) with i' = g*512 + r*128 + p <-> i = g*512+4p+r
    # [B, i, d] -> [B, d, g, p, r] -> [B, d, g, r, p]
    cr = ctx.reshape(B, NG, P, RSLOT, D)
    ctxp = (
        np.transpose(cr[..., : NCH * P], (0, 4, 1, 3, 2))
        .reshape(B, NCH * P, LC)
        .astype(np.float16)
    )

    # tails: [B, 128, g, p]; partition 32r+t holds tail dim t (t<16) or
    # the bias lane (t=16) for slot r; column (g, p) is out row g*512+4p+r
    tails = np.zeros((B, P, NG, P), dtype=np.float16)
    tail_d = np.transpose(cr[..., NCH * P :], (0, 4, 1, 2, 3))  # [B,16,g,p,r]
    for r in range(RSLOT):
        tails[:, 32 * r : 32 * r + 16, :, :] = tail_d[..., r]
        tails[:, 32 * r + 16, :, :] = 1.0

    # R[b, dd, c, j]: blocks 0..2 rows dd -> d = 128c + dd; block 3 rows
    # 32r+t -> tail slice + asp_term row (replicated per slot offset)
    scaled = (asp * w3[None, None, :] + w1[None, None, :]).transpose(0, 2, 1)
    at = asp @ w2
    rr = np.zeros((B, P, NCH + 1, LA), dtype=np.float16)
    for c in range(NCH):
        rr[:, :, c, :] = scaled[:, P * c : P * (c + 1), :]
    for r in range(RSLOT):
        rr[:, 32 * r : 32 * r + 16, NCH, :] = scaled[:, NCH * P :, :]
        rr[:, 32 * r + 16, NCH, :] = at
    return ctxp, tails, rr


def kernel(batch_size=None, ctx=None, asp=None, w_u=None, **_unused):
    ctxp, tails, rr = _prep_inputs(ctx, asp, w_u)

    nc = build_kernel()
    in_maps = [
        {
            "ctx": ctxp[i * PB : (i + 1) * PB],
            "tl": tails[i * PB : (i + 1) * PB],
            "rr": rr[i * PB : (i + 1) * PB],
            # batch-0 group-0 ctx columns, partition-major dup for the head
            "c0": np.ascontiguousarray(
                ctxp[i * PB, :, 0:GROUP]
                .reshape(NCH, P, GROUP)
                .transpose(1, 0, 2)
            ).reshape(P, NCH * GROUP),
        }
        for i in range(N_CORES)
    ]
    res = run_bass_kernel_spmd(
        nc, in_maps, core_ids=list(range(N_CORES)), **_RUN_KWARGS
    )
    _LAST_RESULTS.clear()
    _LAST_RESULTS.append(res)
    # stores write natural row order (partition p, slot r -> row 4p+r)
    out = np.concatenate(
        [np.asarray(res.results[i]["out"]) for i in range(N_CORES)], axis=0
    )
    return out.astype(np.float32)


# test-harness hooks: extra kwargs for run_bass_kernel_spmd (e.g. trace=True)
# and the last BassKernelResults for profiling. Unused in grading.
_RUN_KWARGS: dict = {}
_LAST_RESULTS: list = []
